# revision 36
# baseline (speedup 1.0000x reference)
"""QLoRA-style MLP (NF4 dequant + LoRA + SiLU) on 8 TRN2 NeuronCores.

Sharding: 4-way d_ff (tensor parallel) x 2-way tokens (data parallel).
Each core dequantizes its d_ff shard of both weight matrices on-device
(deg-(3,3) even/odd polynomial f(z) = A(z^2) + z*B(z^2) of the NF4
codebook, split across DVE / ACT / GpSimd), runs fp16 matmuls with fp32
PSUM accumulation, fuses the LoRA terms into the same PSUM groups,
applies SiLU on the ACT engine, and reduces the partial down-projection
outputs across the 4 cores of each token half with on-chip
ReduceScatters (small per-panel collectives that overlap compute; a
1-strip final panel keeps the end-of-kernel RS tail short).

Pipeline structure (what keeps TensorE >88% occupied):
 - DMA rings are specialized: the sync HWDGE ring carries only the
   latency-critical dequant feed (codes + scales), the ACT HWDGE ring
   carries x2/silu traffic, and SWDGE (gpsimd) carries casting loads
   and bulk writes.  This prevents head-of-line blocking of the
   dequant feed at panel boundaries.
 - x tiles are cast fp32->fp16 straight out of the input by SWDGE
   DMAs (no staging pass), quartered for low first-tile latency.
 - absmax scales arrive host-expanded ([2*nt, c] -> [128, nt, c],
   a lossless repeat) so each chunk's scale tile is one contiguous
   casting DMA instead of a slow stride-0 broadcast spray.
 - BOTH weight matrices are dequantized during the up phase: up
   strips stay SBUF-resident; down strips are spilled to a DRAM fp16
   weight cache using idle DVE/ACT/GpSimd slack, then streamed back
   in the down phase as plain deeply-prefetched loads, so the down
   phase has no dequant on its critical path at all.
 - PSUM eviction of the down outputs runs on DVE (idle in the down
   phase), keeping ACT free for the dequant of the next panel.

Host-side work is limited to lossless layout transforms: transpose /
slice / pad / repeat / value-preserving dtype casts.
"""

import numpy as np

import concourse.bass as bass
import concourse.bacc as bacc
import concourse.mybir as mybir
from concourse import bass_utils
from concourse.tile import TileContext

F16 = mybir.dt.float16
F32 = mybir.dt.float32
AF = mybir.ActivationFunctionType
ALU = mybir.AluOpType

# NF4 codebook (bitsandbytes dequantize_4bit)
NF4 = np.array([
    -1.0, -0.6961928009986877, -0.5250730514526367, -0.39491748809814453,
    -0.28444138169288635, -0.18477343022823334, -0.09105003625154495, 0.0,
    0.07958029955625534, 0.16093020141124725, 0.24611230194568634,
    0.33791524171829224, 0.44070982933044434, 0.5626170039176941,
    0.7229568362236023, 1.0], dtype=np.float64)


def _fit_ab():
    """LSQ fit of NF4 over z=(c-7.5)/7.5 as f(z) = A(u) + z*B(u), u=z^2,
    with A and B cubic in u.  Evaluated in fp16 this reproduces the
    table to ~1.3e-3 rms (2.5e-3 relative to the table's rms), which is
    far inside the 2e-2 end-to-end tolerance."""
    c = np.arange(16.0)
    z = (c - 7.5) / 7.5
    u = z * z
    M = np.stack([np.ones(16), u, u**2, u**3, z, z*u, z*u**2, z*u**3], 1)
    coef, *_ = np.linalg.lstsq(M, NF4, rcond=None)
    return [float(v) for v in coef[:4]], [float(v) for v in coef[4:]]


AC, BC = _fit_ab()

# ---------------------------------------------------------------- config

FULL_CFG = dict(
    D=4096,          # d_model
    FSH=2816,        # d_ff shard per core (d_ff padded 11008 -> 11264 = 4*2816)
    TSH=4096,        # tokens per core (8192 = 2*4096)
    TW=512,          # token tile width (one PSUM bank)
    UP_PANELS=[4, 9, 9],    # f-tiles dequantized per up panel (sum = FT)
    DN_PANELS=[4, 9, 9, 6, 4],  # m-tiles per down panel (sum = MT)
    SP_BUFS=11,      # up strip pool depth (panel + lookahead)
    DSP_BUFS=18,     # down strip pool depth (pure DMA stream slots)
    CHT_UP=8,        # dequant chunk, in 128-col tiles, up
    CHT_DN=11,       # dequant chunk, in 128-col tiles, down
    RS_TSPLIT=2,     # ReduceScatter split along tokens per down panel
    R=16,            # lora rank
    N_CORES=8,
    GROUPS=[[0, 1, 2, 3], [4, 5, 6, 7]],
    SILU_VIA_SIGMOID=False,   # True: sigmoid(ACT) + mul(DVE); for CoreSim
)


def _emit_strip(nc, wp, strip, codes_dram, scales16, col_off, nt, cht,
                uid, ctag, negone=None, abias=None, feed_bufs=3,
                out_dram=None):
    """Dequantize one weight strip.

    strip: SBUF tile [128, nt, 128] fp16 (partition = contraction index
    within tile, free = (tile index, output col within tile)).
    codes_dram: DRAM AP [128, nt, 128] fp16 codes for this strip.
    scales16: DRAM tile [2*nt_total, n_cols] fp16; scale for partition p,
    tile k, col w is scales16[2*k + p//64, col_off + w].
    Processes `cht` tiles (cht*128 elements along free) per chunk.

    w = [A(u) + z*B(u)] * scale with z = (c-7.5)/7.5, u = z^2; evaluated
    as Horner in u over g_k = a_k + z*b_k.  Work split: z + Horner (7
    TT) on DVE, u (Square) + g1/g0 on ACT, g3/g2 on GpSimd.
    """
    ch = cht * 128
    sflat = strip.rearrange("p a b -> p (a b)") if strip is not None else None
    for ci, k0 in enumerate(range(0, nt, cht)):
        tag = f"{ctag}{uid}_{ci}"
        cch = wp.tile([128, ch], F16, tag=f"{ctag}cd", bufs=feed_bufs,
                      name=f"cd{tag}")
        nc.sync.dma_start(
            cch.rearrange("p (a b) -> p a b", b=128),
            codes_dram[:, k0:k0 + cht, :])
        scl = wp.tile([128, ch], F16, tag=f"{ctag}sc", bufs=feed_bufs,
                      name=f"sc{tag}")
        scl3 = scl.rearrange("p (a b) -> p a b", b=128)
        # scales arrive host-expanded to [128, nt, ncols] fp32 (lossless
        # repeat); one contiguous casting SWDGE DMA per chunk replaces the
        # slow stride-0 broadcast spray
        nc.gpsimd.dma_start(
            scl3[:], scales16[:, k0:k0 + cht, col_off:col_off + 128])

        sl = slice(k0 * 128, (k0 + cht) * 128)
        z = wp.tile([128, ch], F16, tag=f"{ctag}z", bufs=3, name=f"z{tag}")
        nc.vector.tensor_scalar(z[:], cch[:], -7.5, 1.0 / 7.5, ALU.add,
                                ALU.mult)
        u = wp.tile([128, ch], F16, tag=f"{ctag}u", bufs=3, name=f"u{tag}")
        nc.scalar.activation(u[:], cch[:], AF.Square, bias=negone[:],
                             scale=1.0 / 7.5)
        gs = {}
        for k, eng in ((3, "gp"), (2, "gp"), (1, "act"), (0, "act")):
            g = wp.tile([128, ch], F16, tag=f"{ctag}g", bufs=5,
                        name=f"g{k}{tag}")
            if eng == "gp":
                nc.gpsimd.tensor_scalar(g[:], z[:], BC[k], AC[k], ALU.mult,
                                        ALU.add)
            else:
                nc.scalar.activation(g[:], z[:], AF.Identity,
                                     bias=abias[:, k:k + 1], scale=BC[k])
            gs[k] = g
        h = gs[3]
        for k in (2, 1, 0):
            hm = wp.tile([128, ch], F16, tag=f"{ctag}h", bufs=4,
                         name=f"hm{k}{tag}")
            nc.vector.tensor_mul(hm[:], h[:], u[:])
            ha = wp.tile([128, ch], F16, tag=f"{ctag}h", bufs=4,
                         name=f"ha{k}{tag}")
            nc.vector.tensor_add(ha[:], hm[:], gs[k][:])
            h = ha
        if out_dram is None:
            nc.vector.tensor_mul(sflat[:, sl], h[:], scl[:])
        else:
            # spill mode: dequantized chunk goes to a DRAM weight cache
            # instead of an SBUF-resident strip
            och = wp.tile([128, ch], F16, tag=f"{ctag}o", bufs=2,
                          name=f"o{tag}")
            nc.vector.tensor_mul(och[:], h[:], scl[:])
            nc.gpsimd.dma_start(
                out_dram[:, k0:k0 + cht, :],
                och.rearrange("p (a b) -> p a b", b=128))


def build_nc(cfg):
    D, FSH, TSH, TW = cfg["D"], cfg["FSH"], cfg["TSH"], cfg["TW"]
    R = cfg["R"]
    CHT_UP, CHT_DN = cfg["CHT_UP"], cfg["CHT_DN"]
    KT, FT, MT, TB = D // 128, FSH // 128, D // 128, TSH // TW
    UP_PANELS, DN_PANELS = cfg["UP_PANELS"], cfg["DN_PANELS"]
    assert sum(UP_PANELS) == FT and sum(DN_PANELS) == MT
    G = len(cfg["GROUPS"][0])
    N_RS = len(DN_PANELS)
    MPRS = [(m * 128) // G for m in DN_PANELS]  # output rows per panel
    ROFF = [sum(MPRS[:i]) for i in range(N_RS)]  # y2c row offsets
    NSP = cfg["RS_TSPLIT"]
    TSP = TSH // NSP             # tokens per RS call
    KT2, FT2 = KT // 2, (FT + 1) // 2
    KT4, NXQ = KT // 4, 4

    nc = bacc.Bacc(None, num_devices=cfg["N_CORES"], num_swdge_queues=4)

    xT = nc.dram_tensor("xT", [D, TSH], F32, kind="ExternalInput")
    up_codes = nc.dram_tensor("up_codes", [128, FT, KT, 128], F16,
                              kind="ExternalInput")
    up_scales = nc.dram_tensor("up_scales", [128, KT, FSH], F32,
                               kind="ExternalInput")
    dn_codes = nc.dram_tensor("dn_codes", [128, MT, FT, 128], F16,
                              kind="ExternalInput")
    dn_scales = nc.dram_tensor("dn_scales", [128, FT, D], F32,
                               kind="ExternalInput")
    up_a = nc.dram_tensor("up_a", [128, KT, R], F32, kind="ExternalInput")
    up_b = nc.dram_tensor("up_b", [R, FSH], F32, kind="ExternalInput")
    dn_a = nc.dram_tensor("dn_a", [128, FT, R], F32, kind="ExternalInput")
    dn_b = nc.dram_tensor("dn_b", [R, D], F32, kind="ExternalInput")
    y2c = nc.dram_tensor("y2c", [sum(MPRS), TSH], F32,
                         kind="ExternalOutput")

    with TileContext(nc) as tc:
        with tc.tile_pool(name="dram", bufs=1, space="DRAM") as dram:
            x2T16 = dram.tile([128, TB, FT, TW], F16)
            xT16c = dram.tile([128, KT, TSH], F16)
            dnw16 = dram.tile([128, MT, FT, 128], F16)
            y2p = [dram.tile([DN_PANELS[dp] * 128, TSP], F16,
                             name=f"y2p{dp}_{h}")
                   for dp in range(N_RS) for h in range(NSP)]
            rs_out = [
                dram.tile([MPRS[dp], TSP], F16, name=f"rs{dp}_{h}")
                for dp in range(N_RS) for h in range(NSP)
            ]

            gcp = tc.alloc_tile_pool(name="gconsts", bufs=1)
            negone = gcp.tile([128, 1], F32)
            nc.vector.memset(negone[:], -1.0)
            abias = gcp.tile([128, 4], F32)
            for k in range(4):
                nc.vector.memset(abias[:, k:k + 1], AC[k])

            # ------------------------------------------------ prep phase
            with tc.tile_pool(name="consts", bufs=1) as cp:
                up_a16 = cp.tile([128, KT, R], F16)
                up_b16 = cp.tile([R, FSH], F16)
                tT = cp.tile([R, TSH], F16)
                with tc.tile_pool(name="prep", bufs=2) as pp:
                    a32 = pp.tile([128, KT, R], F32, tag="lora", name="ua32")
                    nc.sync.dma_start(a32[:], up_a[:])
                    nc.vector.tensor_copy(up_a16[:], a32[:])
                    b32 = pp.tile([R, FSH], F32, tag="lorab", name="ub32")
                    nc.sync.dma_start(b32[:], up_b[:])
                    nc.vector.tensor_copy(up_b16[:], b32[:])
                xv = xT.rearrange("(kt ki) t -> ki kt t", ki=128)

                # ---------------------------------------------- up phase
                with (
                    tc.tile_pool(name="ustrip", bufs=cfg["SP_BUFS"]) as sp,
                    tc.tile_pool(name="uwork", bufs=2) as wp,
                    tc.tile_pool(name="ux", bufs=3) as xp,
                    tc.tile_pool(name="ups", bufs=2, space="PSUM") as psp,
                    tc.tile_pool(name="ustage", bufs=3) as stg,
                ):
                    fbase = 0
                    for p, pn in enumerate(UP_PANELS):
                        fts = list(range(fbase, fbase + pn))
                        fbase += pn
                        strips = {}
                        for f in fts:
                            strip = sp.tile([128, KT, 128], F16, tag="ustrip",
                                            bufs=cfg["SP_BUFS"],
                                            name=f"ustrip{f}")
                            cht_f = 4 if (p == 0 and f == fts[0]) else CHT_UP
                            _emit_strip(nc, wp, strip, up_codes[:, f, :, :],
                                        up_scales, 128 * f, KT, cht_f, f, "u",
                                        negone, abias)
                            strips[f] = strip
                        for t in range(TB):
                            tsl = slice(TW * t, TW * (t + 1))
                            xh = []
                            for i in range(NXQ):
                                xt = xp.tile([128, KT4, TW], F16, tag="xt",
                                             bufs=5, name=f"xt{p}_{t}_{i}")
                                ksl = slice(KT4 * i, KT4 * (i + 1))
                                if p == 0:
                                    # fp32 -> fp16 casting SWDGE DMA straight
                                    # from the input, written through to a
                                    # DRAM fp16 cache for the later panels
                                    nc.gpsimd.dma_start(xt[:], xv[:, ksl, tsl])
                                    nc.scalar.dma_start(
                                        xT16c[:, ksl, tsl], xt[:])
                                else:
                                    # plain fp16 reload: half the latency,
                                    # and keeps SWDGE free for scale casts
                                    nc.sync.dma_start(
                                        xt[:], xT16c[:, ksl, tsl])
                                xh.append(xt)
                            if p == 0:
                                pt = psp.tile([R, TW], F32, tag="ptT", bufs=2,
                                              name=f"ptT{t}")
                                for kt in range(KT):
                                    nc.tensor.matmul(pt[:], up_a16[:, kt, :],
                                                     xh[kt // KT4][:, kt % KT4, :],
                                                     start=(kt == 0),
                                                     stop=(kt == KT - 1))
                                nc.vector.tensor_copy(tT[:, tsl], pt[:])
                            for f in fts:
                                ps = psp.tile([128, TW], F32, tag="py1",
                                              bufs=6, name=f"py1_{f}_{t}")
                                for kt in range(KT):
                                    nc.tensor.matmul(
                                        ps[:], strips[f][:, kt, :],
                                        xh[kt // KT4][:, kt % KT4, :],
                                        start=(kt == 0), stop=False)
                                nc.tensor.matmul(
                                    ps[:], up_b16[:, 128 * f:128 * (f + 1)],
                                    tT[:, tsl], start=False, stop=True)
                                so = stg.tile([128, TW], F16, tag="silu",
                                              bufs=2, name=f"so{f}_{t}")
                                if cfg.get("SILU_VIA_SIGMOID"):
                                    sg = stg.tile([128, TW], F16, tag="sg",
                                                  bufs=3, name=f"sg{f}_{t}")
                                    nc.scalar.activation(sg[:], ps[:],
                                                         AF.Sigmoid)
                                    nc.vector.tensor_mul(so[:], ps[:], sg[:])
                                else:
                                    nc.scalar.activation(so[:], ps[:], AF.Silu)
                                nc.scalar.dma_start(x2T16[:, t, f, :], so[:])
                    # dequantize ALL down strips while the up-phase matmul
                    # tail drains (DVE/ACT/GpSimd have slack), spilling the
                    # fp16 weights to DRAM; emitted last = lowest priority,
                    # so it never starves the up-panel dequant
                    for m in range(MT):
                        _emit_strip(nc, wp, None, dn_codes[:, m, :, :],
                                    dn_scales, 128 * m, FT, CHT_DN, f"s{m}",
                                    "u", negone, abias, feed_bufs=3,
                                    out_dram=dnw16[:, m, :, :])

            # ------------------------------------------------ down phase
            with tc.tile_pool(name="dconsts", bufs=1) as dcp:
                dn_a16 = dcp.tile([128, FT, R], F16)
                dn_b16 = dcp.tile([R, D], F16)
                t2T = dcp.tile([R, TSH], F16)
                with tc.tile_pool(name="dprep", bufs=1) as dpp:
                    # scalar-ring DMAs: the sync ring is head-of-line
                    # blocked by dstrip-load triggers at the transition
                    da32 = dpp.tile([128, FT, R], F32, tag="la", name="da32")
                    nc.scalar.dma_start(da32[:], dn_a[:])
                    nc.vector.tensor_copy(dn_a16[:], da32[:])
                    db32 = dpp.tile([R, D], F32, tag="lb", name="db32")
                    nc.scalar.dma_start(db32[:], dn_b[:])
                    nc.vector.tensor_copy(dn_b16[:], db32[:])

                with (
                    tc.tile_pool(name="dstrip", bufs=cfg["DSP_BUFS"]) as dsp,
                    tc.tile_pool(name="dx", bufs=3) as dxp,
                    tc.tile_pool(name="dps", bufs=2, space="PSUM") as dpsp,
                    tc.tile_pool(name="dstage", bufs=3) as dstg,
                ):
                    mbase = 0
                    for dp, pn in enumerate(DN_PANELS):
                        mts = list(range(mbase, mbase + pn))
                        mbase += pn
                        dstrips = {}
                        for m in mts:
                            strip = dsp.tile([128, FT, 128], F16, tag="dstrip",
                                             bufs=cfg["DSP_BUFS"],
                                             name=f"dstrip{m}")
                            # plain deep-prefetched load from the DRAM
                            # weight cache written during the up phase
                            nc.sync.dma_start(strip[:], dnw16[:, m, :, :])
                            dstrips[m] = strip
                        for t in range(TB):
                            tsl = slice(TW * t, TW * (t + 1))
                            x2h = []
                            for i in range(2):
                                fa, fb = FT2 * i, min(FT2 * (i + 1), FT)
                                x2t = dxp.tile([128, FT2, TW], F16, tag="x2t",
                                               bufs=7, name=f"x2t{dp}_{t}_{i}")
                                # the two halves go down different HWDGE
                                # rings so they load in parallel at panel
                                # boundaries
                                eng = nc.scalar if i else nc.sync
                                eng.dma_start(x2t[:, :fb - fa, :],
                                              x2T16[:, t, fa:fb, :])
                                x2h.append(x2t)
                            if dp == 0:
                                pt2 = dpsp.tile([R, TW], F32, tag="pt2",
                                                bufs=2, name=f"pt2_{t}")
                                for ft in range(FT):
                                    nc.tensor.matmul(
                                        pt2[:], dn_a16[:, ft, :],
                                        x2h[ft // FT2][:, ft % FT2, :],
                                        start=(ft == 0), stop=(ft == FT - 1))
                                nc.vector.tensor_copy(t2T[:, tsl], pt2[:])
                            for m in mts:
                                ps = dpsp.tile([128, TW], F32, tag="py2",
                                               bufs=6, name=f"py2_{m}_{t}")
                                for ft in range(FT):
                                    nc.tensor.matmul(
                                        ps[:], dstrips[m][:, ft, :],
                                        x2h[ft // FT2][:, ft % FT2, :],
                                        start=(ft == 0), stop=False)
                                nc.tensor.matmul(
                                    ps[:], dn_b16[:, 128 * m:128 * (m + 1)],
                                    t2T[:, tsl], start=False, stop=True)
                                po = dstg.tile([128, TW], F16, tag="pout",
                                               bufs=8, name=f"po{m}_{t}")
                                nc.vector.tensor_copy(po[:], ps[:])
                                th, tr = t // (TB // NSP), t % (TB // NSP)
                                nc.gpsimd.dma_start(
                                    y2p[NSP * dp + th][
                                        128 * (m - mts[0]):
                                        128 * (m - mts[0] + 1),
                                        TW * tr:TW * (tr + 1)],
                                    po[:])
                        # two token-half ReduceScatters per panel: smaller
                        # collectives overlap the next panel's compute and
                        # shrink the end-of-kernel tail
                        for h in range(NSP):
                            nc.gpsimd.collective_compute(
                                "ReduceScatter",
                                ALU.add,
                                replica_groups=cfg["GROUPS"],
                                ins=[y2p[NSP * dp + h][:].opt()],
                                outs=[rs_out[NSP * dp + h][:].opt()],
                            )
                            # convert this half's reduced shard to fp32 via
                            # a casting SWDGE DMA (DRAM->DRAM, no SBUF bounce,
                            # keeps compute-engine queues out of the RS path)
                            nc.gpsimd.dma_start(
                                y2c[ROFF[dp]:ROFF[dp] + MPRS[dp],
                                    TSP * h:TSP * (h + 1)],
                                rs_out[NSP * dp + h][:])
            gcp.release()
    nc.compile()
    return nc


# ---------------------------------------------------------------- host side

def _tile_codes_k_major(codesT):
    """codesT [K, F] -> [128, F//128, K//128, 128] fp16 (ki, ft, kt, fw)."""
    K, F = codesT.shape
    a = codesT.reshape(K // 128, 128, F // 128, 128)
    return np.ascontiguousarray(a.transpose(1, 2, 0, 3)).astype(np.float16)


def prep_inputs(inputs, cfg):
    D, FSH, TSH, R = cfg["D"], cfg["FSH"], cfg["TSH"], cfg["R"]
    n_cores = cfg["N_CORES"]
    n_ff = len(cfg["GROUPS"][0])
    DFF = inputs["w_up_codes"].shape[0]
    FFP = FSH * n_ff

    x1 = np.asarray(inputs["x1"], np.float32)
    xT_full = np.ascontiguousarray(x1.T)                     # [D, N_TOK]

    upc = np.full((FFP, D), 7, np.int32)
    upc[:DFF] = inputs["w_up_codes"]
    upam = np.ones((FFP, D // 64), np.float32)
    upam[:DFF] = np.asarray(inputs["w_up_absmax"],
                            np.float32).reshape(DFF, D // 64)
    dnc = np.full((D, FFP), 7, np.int32)
    dnc[:, :DFF] = inputs["w_down_codes"]
    dnam = np.ones((D, FFP // 64), np.float32)
    dnam[:, :DFF // 64] = np.asarray(
        inputs["w_down_absmax"], np.float32).reshape(D, DFF // 64)
    upb = np.zeros((R, FFP), np.float32)
    upb[:, :DFF] = inputs["w_up_lora_b"]
    dna = np.zeros((FFP, R), np.float32)
    dna[:DFF] = inputs["w_down_lora_a"]

    up_a_t = np.ascontiguousarray(
        np.asarray(inputs["w_up_lora_a"], np.float32)
        .reshape(D // 128, 128, R).transpose(1, 0, 2))
    dn_b_full = np.ascontiguousarray(
        np.asarray(inputs["w_down_lora_b"], np.float32))

    def _expand_scales(base, nt, ncols):
        # [2*nt, ncols] -> [128, nt, ncols]: out[p, k, c] = base[2k + p//64, c]
        # (pure lossless replication; lets the kernel load scales with
        # plain contiguous DMAs instead of stride-0 broadcast sprays)
        y = base.reshape(nt, 2, ncols)
        return np.ascontiguousarray(
            np.repeat(y, 64, axis=1).transpose(1, 0, 2))

    KT, FT = D // 128, FSH // 128
    in_maps = []
    for c in range(n_cores):
        q, hh = c % n_ff, c // n_ff
        fsl = slice(q * FSH, (q + 1) * FSH)
        bsl = slice(q * (FSH // 64), (q + 1) * (FSH // 64))
        tsl = slice(hh * TSH, (hh + 1) * TSH)
        up_codesT = np.ascontiguousarray(upc[fsl].T)          # [D, FSH]
        dn_codesT = np.ascontiguousarray(dnc[:, fsl].T)       # [FSH, D]
        in_maps.append(dict(
            xT=np.ascontiguousarray(xT_full[:, tsl]),
            up_codes=_tile_codes_k_major(up_codesT),
            up_scales=_expand_scales(upam[fsl].T, KT, FSH),   # [128, KT, FSH]
            dn_codes=_tile_codes_k_major(dn_codesT),
            dn_scales=_expand_scales(dnam[:, bsl].T, FT, D),  # [128, FT, D]
            up_a=up_a_t,
            up_b=np.ascontiguousarray(upb[:, fsl]),
            dn_a=np.ascontiguousarray(
                dna[fsl].reshape(FSH // 128, 128, R).transpose(1, 0, 2)),
            dn_b=dn_b_full,
        ))
    return in_maps


def assemble(outs, cfg):
    D, TSH = cfg["D"], cfg["TSH"]
    n_cores = cfg["N_CORES"]
    n_ff = len(cfg["GROUPS"][0])
    n_t = n_cores // n_ff
    DN_PANELS = cfg["DN_PANELS"]
    MPRS = [(m * 128) // n_ff for m in DN_PANELS]
    N_TOK = TSH * n_t
    y2T = np.zeros((D, N_TOK), np.float32)
    for c in range(n_cores):
        q, hh = c % n_ff, c // n_ff
        out = outs[c]
        roff = 0
        moff = 0
        for dp, mp in enumerate(DN_PANELS):
            mpr = MPRS[dp]
            gm = 128 * moff + mpr * q
            y2T[gm:gm + mpr, TSH * hh:TSH * (hh + 1)] = \
                out[roff:roff + mpr]
            roff += mpr
            moff += mp
    return np.ascontiguousarray(y2T.T)


_NC_CACHE = {}


def kernel(**inputs):
    cfg = FULL_CFG
    if "full" not in _NC_CACHE:
        _NC_CACHE["full"] = build_nc(cfg)
    nc = _NC_CACHE["full"]
    in_maps = prep_inputs(inputs, cfg)
    res = bass_utils.run_bass_kernel_spmd(
        nc, in_maps, core_ids=list(range(cfg["N_CORES"])))
    return assemble([r["y2c"] for r in res.results], cfg)

